# revision 29
# baseline (speedup 1.0000x reference)
"""Cross-attention kernel for Trainium2 (8 NeuronCores).

Problem (reference semantics, all fp32):
    q = split_heads(dec @ q_w + q_b)        # [B,H,Sq,64]
    k = split_heads(enc @ k_w + k_b)        # [B,H,Sk,64]
    v = split_heads(enc @ v_w + v_b)        # [B,H,Sk,64]
    a = softmax(mask(q k^T / 8))
    out = merge_heads(a @ v) @ o_w + o_b    # [B,Sq,1024]
with B=4, Sq=1024, Sk=2048, D=1024, H=16.

Sharding: batch x head-group. Core c handles batch b=c//2 and heads
hg*8..hg*8+8 (hg=c%2), i.e. a 512-wide feature slice of the q/k/v
projections and the matching 512 rows of o_w. Each core emits a full
[1024,1024] partial of its batch's output; the host sums the two
partials per batch and adds o_b.

Host prep: dec/enc are fed PRE-TRANSPOSED ([d, seq]) and cast to bf16,
as are the weight slices — no PE transposes on device and half the HBM
traffic. Projections run in bf16 (PSUM accumulates fp32); the attention
core (scores exp / PV) runs with bf16 scores operands and fp32r PV so
the softmax path keeps fp32 headroom.

Masked keys are also COMPACTED host-side: enc keeps only unmasked keys
(padded to a whole number of 128-key tiles, pad keys get the -1e30 exp
bias), so the usual ~10% masked fraction drops one whole key tile (15
instead of 16); a 16-tile variant is compiled lazily as fallback.

On-core dataflow:
  qT[ft]=[128f,1024q] bf16 and kT[ft]=[128f,sk] bf16 from chained bf16
  matmuls with the bias fused in the PSUM->SBUF activation. v is
  produced natural-layout [k,f] with the bias folded in as a rank-1
  ones matmul, stored f32r with a ones column appended per head (so the
  PV matmul also produces the softmax denominator Z in row 64).
  Attention runs one head at a time, paced by the Scalar engine's exp
  (~1.14us per [128,1024] tile, the hard floor):
    - sc psum tiles are triple-buffered so scores[c] only WAR-waits on
      exp[c-3];
    - PV lags scores by two slots so the in-order PE queue never
      reaches a PV before its exp finished (a one-slot lag phase-locks
      the loop into a serial ~2us cycle);
    - heads 1-7 are ONE continuous software pipeline (the PV lag queue
      flows across head boundaries);
    - head 0 additionally emits the v-projection chain for key-tile c
      in its slot c, so v-production rides under its (PE-paced) pass;
    - the exp/xu/z pools are allocated at LOW SBUF addresses: with the
      ex tiles high in SBUF the Scalar exp measurably slows ~20%.
  Normalization copies xp to SBUF (the copy is xp's only reader, so
  PSUM frees after one DVE op), then reciprocal_approx_fast(Z) +
  partition_broadcast + multiply, writing xT bf16, which the
  o-projection consumes as the stationary operand. Output partials are
  written bf16 and summed on host in fp32.
"""
import numpy as np

P = 128
B, S_ENC, S_DEC, D, H = 4, 2048, 1024, 1024, 16
HD = D // H                     # 64
NCORES = 8
FSH = 512                       # features per core (8 heads x 64)
HPC = 8                         # heads per core
NQT = S_DEC // P                # 8
NET = D // P                    # 8
NST = S_ENC // P                # 16
NFT = FSH // P                  # 4
VG = HD + 1                     # 65: v cols per head incl ones column

_NC = {}


def _build_nc(nstk=NST):
    from contextlib import ExitStack
    import concourse.bass as bass
    import concourse.tile as tile
    from concourse import bacc, mybir

    F32 = mybir.dt.float32
    F32R = mybir.dt.float32r
    BF16 = mybir.dt.bfloat16
    ts = bass.ts
    Ident = mybir.ActivationFunctionType.Identity
    Exp = mybir.ActivationFunctionType.Exp

    sk = nstk * P
    nc = bacc.Bacc("TRN2", target_bir_lowering=False, debug=False)

    decT = nc.dram_tensor("decT", [D, S_DEC], BF16, kind="ExternalInput").ap()
    encT = nc.dram_tensor("encT", [D, sk], BF16, kind="ExternalInput").ap()
    qw = nc.dram_tensor("qw", [D, FSH], BF16, kind="ExternalInput").ap()
    kw = nc.dram_tensor("kw", [D, FSH], BF16, kind="ExternalInput").ap()
    vw = nc.dram_tensor("vw", [D, FSH], BF16, kind="ExternalInput").ap()
    ow = nc.dram_tensor("ow", [FSH, D], BF16, kind="ExternalInput").ap()
    qb = nc.dram_tensor("qb", [P, NFT], F32, kind="ExternalInput").ap()
    kb = nc.dram_tensor("kb", [P, NFT], F32, kind="ExternalInput").ap()
    vb = nc.dram_tensor("vb", [1, FSH], BF16, kind="ExternalInput").ap()
    maskb = nc.dram_tensor("maskb", [P, nstk], F32, kind="ExternalInput").ap()
    ones1 = nc.dram_tensor("ones1", [1, P], BF16, kind="ExternalInput").ap()
    onescol = nc.dram_tensor("onescol", [P, HPC], F32R, kind="ExternalInput").ap()
    outp = nc.dram_tensor("outp", [S_DEC, D], BF16, kind="ExternalOutput").ap()

    with tile.TileContext(nc) as tc, ExitStack() as ctx:
        const = ctx.enter_context(tc.tile_pool(name="const", bufs=1))
        qb_t = const.tile([P, NFT], F32, tag="qb")
        kb_t = const.tile([P, NFT], F32, tag="kb")
        maskb_t = const.tile([P, nstk], F32, tag="maskb")
        vb_t = const.tile([1, FSH], BF16, tag="vb")
        ones1_t = const.tile([1, P], BF16, tag="ones1")
        onescol_t = const.tile([P, HPC], F32R, tag="onescol")
        for t, src in ((qb_t, qb), (kb_t, kb), (maskb_t, maskb),
                       (vb_t, vb), (ones1_t, ones1), (onescol_t, onescol)):
            nc.sync.dma_start(t[:], src[:])

        # attention working pools allocated FIRST: low SBUF addresses keep
        # the Scalar exp write stream off whatever banks the hot matmul
        # operands live in (measured +220ns/exp when ex sits high).
        expp = ctx.enter_context(tc.tile_pool(name="expp", bufs=5))
        ex_tiles = [expp.tile([P, S_DEC], F32R, tag="ex", name=f"exr{i}")
                    for i in range(5)]
        xup = ctx.enter_context(tc.tile_pool(name="xup", bufs=2))
        zp = ctx.enter_context(tc.tile_pool(name="zp", bufs=2))

        persist = ctx.enter_context(tc.tile_pool(name="persist", bufs=1))
        qT = [persist.tile([P, S_DEC], BF16, tag=f"qT{t}", name=f"qT{t}")
              for t in range(NFT)]
        kT = [persist.tile([P, sk], BF16, tag=f"kT{t}", name=f"kT{t}")
              for t in range(NFT)]
        vt = [persist.tile([P, HPC * VG], F32R, tag=f"v{t}", name=f"v{t}")
              for t in range(nstk)]
        xT = [persist.tile([P, S_DEC], BF16, tag=f"xT{t}", name=f"xT{t}")
              for t in range(NFT)]
        ow_t = [persist.tile([P, D], BF16, tag=f"ow{t}", name=f"ow{t}")
                for t in range(NFT)]

        # ones column per head in the augmented v tiles (engine copies, not
        # DMA: strided 4B-descriptor DMAs would stall the load queue)
        for t in range(nstk):
            dst = vt[t][:].rearrange("p (h c) -> p h c", h=HPC, c=VG)[:, :, HD:VG]
            nc.gpsimd.tensor_copy(dst, onescol_t[:])

        # ---- stage 1: qT = qw^T @ decT + qb ------------------------------
        with tc.tile_pool(name="s1w", bufs=1) as s1w, \
             tc.tile_pool(name="pqs", bufs=4, space="PSUM") as pqs:
            qw_t = [s1w.tile([P, FSH], BF16, tag=f"qw{j}", name=f"qw{j}")
                    for j in range(NET)]
            dct = [s1w.tile([P, S_DEC], BF16, tag=f"dc{j}", name=f"dc{j}")
                   for j in range(NET)]
            for j in range(NET):
                nc.sync.dma_start(qw_t[j][:], qw[ts(j, P), :])
                nc.sync.dma_start(dct[j][:], decT[ts(j, P), :])
            for ft in range(NFT):
                for g in range(2):
                    pq = pqs.tile([P, 512], F32, tag="pq")
                    for j in range(NET):
                        nc.tensor.matmul(pq[:], qw_t[j][:, ts(ft, P)],
                                         dct[j][:, ts(g, 512)],
                                         start=(j == 0), stop=(j == NET - 1))
                    nc.scalar.activation(qT[ft][:, ts(g, 512)], pq[:], Ident,
                                         bias=qb_t[:, ft:ft + 1])

        # ---- stage 2: kT only (v rides under head 0's attention) ---------
        vw_t = [persist.tile([P, FSH], BF16, tag=f"vw{j}", name=f"vw{j}")
                for j in range(NET)]
        ect = [persist.tile([P, sk], BF16, tag=f"ec{j}", name=f"ec{j}")
               for j in range(NET)]
        with tc.tile_pool(name="s2w", bufs=1) as s2w, \
             tc.tile_pool(name="pks", bufs=4, space="PSUM") as pks:
            kw_t = [s2w.tile([P, FSH], BF16, tag=f"kw{j}", name=f"kw{j}")
                    for j in range(NET)]
            for j in range(NET):
                nc.sync.dma_start(kw_t[j][:], kw[ts(j, P), :])
                nc.sync.dma_start(ect[j][:], encT[ts(j, P), :])
            for j in range(NET):
                nc.sync.dma_start(vw_t[j][:], vw[ts(j, P), :])
            kchunks = []
            off = 0
            while off < sk:
                w = min(512, sk - off)
                kchunks.append((off, w))
                off += w
            for ft in range(NFT):
                for off, w in kchunks:
                    pk = pks.tile([P, 512], F32, tag="pk")
                    for j in range(NET):
                        nc.tensor.matmul(pk[:, 0:w], kw_t[j][:, ts(ft, P)],
                                         ect[j][:, off:off + w],
                                         start=(j == 0), stop=(j == NET - 1))
                    nc.scalar.activation(kT[ft][:, off:off + w], pk[:, 0:w],
                                         Ident, bias=kb_t[:, ft:ft + 1])

        # issue o-weights load early so it hides under attention
        for t in range(NFT):
            nc.sync.dma_start(ow_t[t][:], ow[ts(t, P), :])

        # ---- stage 3: attention ------------------------------------------
        # Head 0 is PE-paced: each of its key-tile slots also emits the
        # v-projection chain for that tile, so v-production rides under the
        # scores->exp->PV pipeline. Heads 1-7 are Scalar(exp)-paced with sc
        # triple-buffered to keep the WAR on exp off the critical path.
        if True:

            def attn_head(h, scp, xpp, pre_slot):
                p, r0 = h // 2, (h % 2) * HD
                xp = xpp.tile([VG, S_DEC], F32, tag="xp", name=f"xp{h}")
                pend = []

                def pv(pc, pex):
                    for qh in range(2):
                        nc.tensor.matmul(xp[:, ts(qh, 512)],
                                         vt[pc][:, h * VG:(h + 1) * VG],
                                         pex[:, ts(qh, 512)],
                                         start=(pc == 0),
                                         stop=(pc == nstk - 1))
                for c in range(nstk):
                    if pre_slot is not None:
                        pre_slot(c)
                    sc = scp.tile([P, S_DEC], F32, tag="sc")
                    for qh in range(2):
                        nc.tensor.matmul(sc[:, ts(qh, 512)],
                                         kT[p][r0:r0 + HD, ts(c, P)],
                                         qT[p][r0:r0 + HD, ts(qh, 512)],
                                         start=True, stop=True)
                    # PV lags scores by TWO slots so the in-order PE queue
                    # never reaches a PV before its exp has finished -- a
                    # one-slot lag phase-locks into the serial ~2us cycle.
                    if len(pend) == 2:
                        pv(*pend.pop(0))
                    ex = expp.tile([P, S_DEC], F32R, tag="ex")
                    nc.scalar.activation(ex[:], sc[:], Exp,
                                         bias=maskb_t[:, c:c + 1],
                                         scale=0.125)
                    pend.append((c, ex))
                for pc, pex in pend:
                    pv(pc, pex)
                xu = xup.tile([VG, S_DEC], F32, tag="xu")
                nc.vector.tensor_copy(xu[:], xp[:])
                zin = zp.tile([1, S_DEC], F32, tag="zin")
                nc.vector.tensor_copy(zin[:], xu[HD:VG, :])
                zrec = zp.tile([1, S_DEC], F32, tag="zrec")
                nc.vector.reciprocal_approx_fast(zrec[:], zin[:])
                zbs = zp.tile([HD, S_DEC], F32, tag="zbs")
                nc.gpsimd.partition_broadcast(zbs[:], zrec[:])
                nc.vector.tensor_mul(xT[p][r0:r0 + HD, :],
                                     xu[0:HD, :], zbs[:])

            xpp_shared = ctx.enter_context(
                tc.tile_pool(name="xpp", bufs=1, space="PSUM"))
            with tc.tile_pool(name="scp0", bufs=2, space="PSUM") as scp0, \
                 tc.tile_pool(name="pvp", bufs=2, space="PSUM") as pvp:
                def vchain(c):
                    pv = pvp.tile([P, 512], F32, tag="pv")
                    for j in range(NET):
                        nc.tensor.matmul(pv[:], ect[j][:, ts(c, P)],
                                         vw_t[j][:],
                                         start=(j == 0), stop=False)
                    nc.tensor.matmul(pv[:], ones1_t[:], vb_t[:],
                                     start=False, stop=True)
                    dst = vt[c][:].rearrange(
                        "p (h c) -> p h c", h=HPC, c=VG)[:, :, 0:HD]
                    nc.vector.tensor_copy(
                        dst, pv[:].rearrange("p (h c) -> p h c", h=HPC, c=HD))
                attn_head(0, scp0, xpp_shared, vchain)
            # Heads 1-7 run as ONE continuous pipeline: the PV lag queue
            # flows across head boundaries so neither engine sees a refill
            # bubble; a head's normalize is emitted when its last PV pops.
            with tc.tile_pool(name="scp", bufs=3, space="PSUM") as scp:
                xpp = xpp_shared
                xp_by_h = {}
                pend = []

                def pv_flat(ph, pc, pex):
                    for qh in range(2):
                        nc.tensor.matmul(xp_by_h[ph][:, ts(qh, 512)],
                                         vt[pc][:, ph * VG:(ph + 1) * VG],
                                         pex[:, ts(qh, 512)],
                                         start=(pc == 0),
                                         stop=(pc == nstk - 1))
                    if pc == nstk - 1:
                        normalize(ph, xp_by_h.pop(ph))

                def normalize(ph, xp):
                    pp, rr = ph // 2, (ph % 2) * HD
                    xu = xup.tile([VG, S_DEC], F32, tag="xu")
                    nc.vector.tensor_copy(xu[:], xp[:])
                    zin = zp.tile([1, S_DEC], F32, tag="zin")
                    nc.vector.tensor_copy(zin[:], xu[HD:VG, :])
                    zrec = zp.tile([1, S_DEC], F32, tag="zrec")
                    nc.vector.reciprocal_approx_fast(zrec[:], zin[:])
                    zbs = zp.tile([HD, S_DEC], F32, tag="zbs")
                    nc.gpsimd.partition_broadcast(zbs[:], zrec[:])
                    nc.vector.tensor_mul(xT[pp][rr:rr + HD, :],
                                         xu[0:HD, :], zbs[:])

                for s in range((HPC - 1) * nstk):
                    h, c = 1 + s // nstk, s % nstk
                    p, r0 = h // 2, (h % 2) * HD
                    if c == 0:
                        xp_by_h[h] = xpp.tile([VG, S_DEC], F32, tag="xp",
                                              name=f"xp{h}")
                    sc = scp.tile([P, S_DEC], F32, tag="sc")
                    for qh in range(2):
                        nc.tensor.matmul(sc[:, ts(qh, 512)],
                                         kT[p][r0:r0 + HD, ts(c, P)],
                                         qT[p][r0:r0 + HD, ts(qh, 512)],
                                         start=True, stop=True)
                    if len(pend) == 2:
                        pv_flat(*pend.pop(0))
                    ex = expp.tile([P, S_DEC], F32R, tag="ex")
                    nc.scalar.activation(ex[:], sc[:], Exp,
                                         bias=maskb_t[:, c:c + 1],
                                         scale=0.125)
                    pend.append((h, c, ex))
                for args in pend:
                    pv_flat(*args)

        # ---- stage 4: o-projection ---------------------------------------
        with tc.tile_pool(name="outpool", bufs=4) as outpool, \
             tc.tile_pool(name="pops", bufs=6, space="PSUM") as pops:
            for qt in range(NQT):
                for gh in range(2):
                    po = pops.tile([P, 512], F32, tag="po")
                    for fc in range(NFT):
                        nc.tensor.matmul(po[:], xT[fc][:, ts(qt, P)],
                                         ow_t[fc][:, ts(gh, 512)],
                                         start=(fc == 0), stop=(fc == NFT - 1))
                    ot = outpool.tile([P, 512], BF16, tag="ot")
                    if gh == 0:
                        nc.vector.tensor_copy(ot[:], po[:])
                    else:
                        nc.scalar.copy(ot[:], po[:])
                    nc.sync.dma_start(outp[ts(qt, P), ts(gh, 512)], ot[:])

    nc.compile()
    return nc


def _get_nc(nstk=NST):
    if nstk not in _NC:
        _NC[nstk] = _build_nc(nstk)
    return _NC[nstk]


def _nstk_for(enc_mask):
    """Compacted key-tile count: masked keys are dropped host-side, so most
    masks fit 15 tiles (1920 kept keys); fall back to 16 if one doesn't."""
    kept = (~np.asarray(enc_mask[:, 0, 0], dtype=bool)).sum(axis=1)
    return NST - 1 if int(kept.max()) <= (NST - 1) * P else NST


def make_in_maps(enc, enc_mask, dec, q_w, q_b, k_w, k_b, v_w, v_b, o_w, o_b,
                 nstk=None):
    import ml_dtypes
    bf16 = ml_dtypes.bfloat16
    f32 = np.float32
    ca = np.ascontiguousarray
    enc = np.asarray(enc, dtype=f32)
    dec = np.asarray(dec, dtype=f32)
    mask = np.asarray(enc_mask[:, 0, 0], dtype=bool)
    if nstk is None:
        nstk = _nstk_for(enc_mask)
    sk = nstk * P
    in_maps = []
    for c in range(NCORES):
        b, hg = c // 2, c % 2
        fs = slice(hg * FSH, (hg + 1) * FSH)
        keep = ~mask[b]
        nk = int(keep.sum())
        enc_c = np.zeros((sk, D), dtype=f32)
        enc_c[:nk] = enc[b][keep]
        mb = np.full(sk, f32(-1e30))
        mb[:nk] = 0.0
        in_maps.append({
            "decT": ca(dec[b].T).astype(bf16),
            "encT": ca(enc_c.T).astype(bf16),
            "qw": ca(np.asarray(q_w[:, fs], dtype=f32)).astype(bf16),
            "kw": ca(np.asarray(k_w[:, fs], dtype=f32)).astype(bf16),
            "vw": ca(np.asarray(v_w[:, fs], dtype=f32)).astype(bf16),
            "ow": ca(np.asarray(o_w[fs, :], dtype=f32)).astype(bf16),
            "qb": ca(np.asarray(q_b[fs], dtype=f32).reshape(NFT, P).T),
            "kb": ca(np.asarray(k_b[fs], dtype=f32).reshape(NFT, P).T),
            "vb": ca(np.asarray(v_b[fs], dtype=f32).reshape(1, FSH)).astype(bf16),
            "maskb": ca(mb.reshape(nstk, P).T),
            "ones1": np.ones((1, P), dtype=bf16),
            "onescol": np.ones((P, HPC), dtype=f32),
        })
    return in_maps


def assemble(results, o_b):
    out = np.empty((B, S_DEC, D), dtype=np.float32)
    for b in range(B):
        out[b] = (np.asarray(results[2 * b]["outp"], dtype=np.float32)
                  + np.asarray(results[2 * b + 1]["outp"], dtype=np.float32)
                  + np.asarray(o_b, dtype=np.float32))
    return out


def kernel(enc, enc_mask, dec, q_w, q_b, k_w, k_b, v_w, v_b, o_w, o_b):
    from concourse.bass_utils import run_bass_kernel_spmd
    nstk = _nstk_for(np.asarray(enc_mask))
    nc = _get_nc(nstk)
    in_maps = make_in_maps(enc, enc_mask, dec, q_w, q_b, k_w, k_b,
                           v_w, v_b, o_w, o_b, nstk=nstk)
    res = run_bass_kernel_spmd(nc, in_maps, list(range(NCORES)))
    return assemble(res.results, o_b)
